# revision 16
# baseline (speedup 1.0000x reference)
"""DeepSet (segment_reduce) Trainium2 Bass kernel, v2.

Computes, for each batch row b of x [B, 544]:
    s_i = x[:, :16]; s_g = x[:, 16:32]; s_js = x[:, 32:].reshape(B, 32, 16)
    h   = relu(s_js @ W0 + b0); h = relu(h @ W1 + b1); h = h @ W2 + b2
    summ = h.sum(axis=1)
    out = relu([s_i, s_g, summ] @ RW0 + rb0) @ RW1 + rb1        # [B, 16]

Sharding: pure data-parallel over 8 NeuronCores (batch 16384 -> 8 x 2048),
weights replicated.

v2 structure (vs the v1 transpose-on-chip design):
- x is transposed on the HOST: neighbor features arrive as xs8 [512, 2048]
  fp8e4m3 (slab-major feature rows x batch cols) and s_i/s_g as xg [32, 2048]
  f32. No PE transposes, no PSUM->SBUF x copies.
- L0 runs as fp8 DoubleRow matmuls (0.5 PE cycles/row): stationary packs
  [q8(8*W0var) | q8-residual] planes, the moving slab is a stride-0
  broadcast, so the residual plane corrects the weight quantization for
  free. h0 PSUM = 8*(W0^T x8); biases are pre-scaled 8x on the host and the
  8x is divided out at the summ drain (exact power-of-two arithmetic).
- L1/L2/rho stay float32r (full accuracy; L2 neighbor-sum fused via PSUM
  accumulation).
- The two big PSUM->SBUF relu drains per pair are split greedily between
  ACT and DVE (Pool/GPSIMD cannot access PSUM on TRN2).
"""

import numpy as np
import ml_dtypes
from contextlib import ExitStack

F8NP = ml_dtypes.float8_e4m3

STATE_DIM = 16
N_NEIGH = 32
HIDDEN = 64
XCOLS = (2 + N_NEIGH) * STATE_DIM  # 544
B_FULL = 16384
N_CORES = 8
BC = B_FULL // N_CORES  # 2048 rows per core
SB = 512                # batch rows per super-block (matmul N)
NSB_FULL = BC // SB     # 4
W0SCALE = 8.0

_CACHE = {}

_WOFF_C = {
    "b0s": (0, 1, 128),
    "b1s": (1, 2, 128),
    "b2s": (2, 3, HIDDEN),
    "rb0a": (3, 4, 128),
    "rb0b": (4, 5, 128),
    "rb1": (5, 6, 16),
    "ident": (6, 22, 16),
}
WC_COLS = 22
_WOFF_D = {
    "w1": (0, 128, 128),
    "w2": (128, 192, 128),
    "rw0a": (192, 320, 96),
    "rw0b": (320, 448, 96),
    "rw1a": (448, 464, 128),
    "rw1b": (464, 480, 128),
}
WD_COLS = 480


def build_nc(n_sb=NSB_FULL):
    import concourse.bass as bass
    import concourse.bacc as bacc
    import concourse.tile as tile
    import concourse.mybir as mybir

    f32 = mybir.dt.float32
    f32r = mybir.dt.float32r
    f8 = mybir.dt.float8e4
    AF = mybir.ActivationFunctionType
    ALU = mybir.AluOpType
    DR = mybir.MatmulPerfMode.DoubleRow

    rows = n_sb * SB
    n_blocks = rows // 128
    nc = bacc.Bacc("TRN2", target_bir_lowering=False, debug=False)

    xs8 = nc.declare_dram_parameter("xs8", [512, rows], f8, isOutput=False)
    xg = nc.declare_dram_parameter("xg", [32, rows], f32, isOutput=False)
    wA = nc.declare_dram_parameter("wA", [128, 2048], f8, isOutput=False)
    wC = nc.declare_dram_parameter("wC", [128, WC_COLS], f32, isOutput=False)
    wD = nc.declare_dram_parameter("wD", [128, WD_COLS], f32r, isOutput=False)
    y = nc.declare_dram_parameter("y", [rows, 16], f32, isOutput=True)

    with tile.TileContext(nc) as tc, ExitStack() as ctx:
        wp = ctx.enter_context(tc.tile_pool(name="wts", bufs=1))
        # DMA-written tiles get dedicated slots (single-sync-wait rule).
        pxs = ctx.enter_context(tc.tile_pool(name="xs", bufs=4 * n_sb))
        ph0 = ctx.enter_context(tc.tile_pool(name="h0", bufs=6))
        pr1 = ctx.enter_context(tc.tile_pool(name="r1", bufs=6))
        pX96 = ctx.enter_context(tc.tile_pool(name="X96", bufs=n_sb))
        pr0 = ctx.enter_context(tc.tile_pool(name="r0", bufs=n_sb))
        poT = ctx.enter_context(tc.tile_pool(name="oT", bufs=n_sb))
        poN = ctx.enter_context(tc.tile_pool(name="oN", bufs=1))
        pA = ctx.enter_context(tc.tile_pool(name="pA", bufs=3, space="PSUM"))
        qAcc = ctx.enter_context(tc.tile_pool(name="qAcc", bufs=2, space="PSUM"))

        twC = wp.tile([128, WC_COLS], f32, tag="wC")
        dmaC = nc.sync.dma_start(twC[:], wC[:])
        twA0 = wp.tile([128, 2048], f8, tag="wA0")
        dmaA = nc.sync.dma_start(twA0[:], wA[:])

        xst = {}

        def load_xs(sb):
            for j in range(4):
                t = pxs.tile([128, SB], f8, tag="xs", name=f"xs{sb}_{j}")
                nc.sync.dma_start(
                    t[:], xs8[128 * j:128 * (j + 1), SB * sb:SB * (sb + 1)])
                xst[(sb, j)] = t

        twD = wp.tile([128, WD_COLS], f32r, tag="wD")
        dmaD = nc.sync.dma_start(twD[:], wD[:])
        load_xs(0)
        if n_sb > 1:
            load_xs(1)
        txg = wp.tile([32, rows], f32, tag="xg")
        dmaG = nc.sync.dma_start(txg[:], xg[:])
        for sb in range(2, n_sb):
            load_xs(sb)

        def wc(name):
            c0, c1, p = _WOFF_C[name]
            return twC[0:p, c0:c1]

        def wd(name):
            c0, c1, p = _WOFF_D[name]
            return twD[0:p, c0:c1]

        def w0var(m):  # [128, 2, 128] fp8 hi|res planes of 8*W0 variant m
            return twA0[:, 256 * m:256 * (m + 1)].rearrange(
                "p (two c) -> p two c", two=2)

        tw1, tw2 = wd("w1"), wd("w2")
        trw0a, trw0b = wd("rw0a"), wd("rw0b")
        trw1a, trw1b = wd("rw1a"), wd("rw1b")
        tb0, tb1, tb2s = wc("b0s"), wc("b1s"), wc("b2s")
        trb0a, trb0b, trb1 = wc("rb0a"), wc("rb0b"), wc("rb1")
        tid = wc("ident")

        # Single-sync-wait discipline: each engine observes the startup DMAs
        # it depends on through dummy single-wait ops before real work.
        prev = {"pe": None, "act": None, "dve": None}

        def observe(k, ins):
            if prev[k] is not None:
                tile.add_dep_helper(ins.ins, prev[k].ins, sync=False,
                                    reason="startup order")
            prev[k] = ins

        dq0 = qAcc.tile([1, 1], f32, tag="qAcc")
        observe("pe", nc.tensor.matmul(
            dq0[0:1, 0:1], twC[0:1, 0:1], twC[0:1, 0:1],
            start=True, stop=True))
        dq2 = qAcc.tile([1, 1], f32, tag="qAcc")
        observe("pe", nc.tensor.matmul(
            dq2[0:1, 0:1], twA0[0:1, 0:4].bitcast(f32),
            twA0[0:1, 0:4].bitcast(f32), start=True, stop=True))
        da0 = wp.tile([1, 1], f32, tag="dumA0")
        observe("act", nc.scalar.copy(da0[0:1, 0:1], twC[0:1, 0:1]))
        dv0 = wp.tile([1, 1], f32, tag="dumV0")
        observe("dve", nc.vector.tensor_copy(dv0[0:1, 0:1], twC[0:1, 0:1]))

        # Greedy ACT/DVE load balance for the big PSUM relu drains.
        load = {"act": 0.0, "dve": 0.0}

        def drain_relu(dst, src, bias, eng=None):
            n = src.shape[-1]
            c_act = n / 1.2 + 370
            c_dve = n / 0.92 + 250
            if eng is None:
                eng = ("act" if load["act"] + c_act <= load["dve"] + c_dve
                       else "dve")
            if eng == "act":
                load["act"] += c_act
                observe("act", nc.scalar.activation(
                    dst, src, AF.Relu, bias=bias))
            else:
                load["dve"] += c_dve
                observe("dve", nc.vector.tensor_scalar(
                    dst, src, bias, 0.0, ALU.add, ALU.max))

        n_pairs = 16 * n_sb
        state = {}
        sbs = {}
        deferred = []
        tail = {}
        oN = poN.tile([128, 16 * n_blocks], f32, tag="oN")

        def emit_l0(p):
            sb = p // 16
            n0 = 2 * (p % 16)
            if p % 16 == 0:
                X96 = pX96.tile([96, SB], f32r, tag="X96", name=f"X96_{sb}")
                sbs[sb] = {"X96": X96, "acc": None}
            hp = pA.tile([128, 1024], f32, tag="pA")
            for k in (0, 1):
                n = n0 + k
                observe("pe", nc.tensor.matmul(
                    hp[:, 512 * k:512 * (k + 1)],
                    w0var(n % 8),
                    xst[(sb, n // 8)][:].unsqueeze(1).broadcast_to(
                        [128, 2, SB]),
                    start=True, stop=True, perf_mode=DR,
                ))
            h0s = ph0.tile([128, 1024], f32r, tag="h0")
            drain_relu(h0s[:], hp[:], tb0[:, 0:1])
            state[p] = {"h0s": h0s, "hp": hp}

        def emit_l1(p):
            if p == 0:
                dq1 = qAcc.tile([1, 1], f32, tag="qAcc")
                observe("pe", nc.tensor.matmul(
                    dq1[0:1, 0:1], twD[0:1, 0:1].bitcast(f32),
                    twD[0:1, 0:1].bitcast(f32), start=True, stop=True))
            h0s = state[p]["h0s"]
            h1p = state[p]["hp"]  # reuse: WAR == the existing h0s RAW dep
            for k in (0, 1):
                observe("pe", nc.tensor.matmul(
                    h1p[:, 512 * k:512 * (k + 1)], tw1,
                    h0s[:, 512 * k:512 * (k + 1)],
                    start=True, stop=True,
                ))
            r1 = pr1.tile([128, 1024], f32r, tag="r1")
            drain_relu(r1[:], h1p[:], tb1[:, 0:1])
            state[p]["r1"] = r1

        def emit_l2(p):
            sb = p // 16
            n0 = 2 * (p % 16)
            if sbs[sb]["acc"] is None:
                sbs[sb]["acc"] = qAcc.tile([HIDDEN, SB], f32, tag="qAcc",
                                           name=f"acc{sb}")
            acc = sbs[sb]["acc"]
            r1 = state[p]["r1"]
            for k in (0, 1):
                n = n0 + k
                observe("pe", nc.tensor.matmul(
                    acc[:], tw2, r1[:, 512 * k:512 * (k + 1)],
                    start=(n == 0), stop=(n == N_NEIGH - 1),
                ))
            del state[p]

        def emit_rho(sb):
            d = sbs[sb]

            def s_sg():
                load["act"] += SB / 1.2 + 370
                observe("act", nc.scalar.copy(
                    d["X96"][64:96, :], txg[:, SB * sb:SB * (sb + 1)]))

            def s_summ():
                load["act"] += SB / 1.2 + 370
                observe("act", nc.scalar.activation(
                    d["X96"][0:64, :], d["acc"][:], AF.Identity,
                    bias=tb2s[:, 0:1], scale=1.0 / W0SCALE))

            def s_rho0():
                d["r0p"] = pA.tile([128, 1024], f32, tag="pA", name=f"r0p{sb}")
                observe("pe", nc.tensor.matmul(
                    d["r0p"][:, 0:512], trw0a, d["X96"][:],
                    start=True, stop=True))
                observe("pe", nc.tensor.matmul(
                    d["r0p"][:, 512:1024], trw0b, d["X96"][:],
                    start=True, stop=True))

            def s_relu0():
                d["r0s"] = pr0.tile([128, 1024], f32r, tag="r0",
                                    name=f"r0s{sb}")
                observe("act", nc.scalar.activation(
                    d["r0s"][:, 0:512], d["r0p"][:, 0:512], AF.Relu,
                    bias=trb0a[:, 0:1]))
                observe("dve", nc.vector.tensor_scalar(
                    d["r0s"][:, 512:1024], d["r0p"][:, 512:1024],
                    trb0b[:, 0:1], 0.0, ALU.add, ALU.max))

            def s_rho1():
                d["op"] = pA.tile([16, SB], f32, tag="pA", name=f"op{sb}")
                observe("pe", nc.tensor.matmul(
                    d["op"][:], trw1a, d["r0s"][:, 0:512],
                    start=True, stop=False))
                observe("pe", nc.tensor.matmul(
                    d["op"][:], trw1b, d["r0s"][:, 512:1024],
                    start=False, stop=True))

            def s_bias():
                d["oT"] = poT.tile([16, SB], f32, tag="oT", name=f"oT{sb}")
                load["act"] += SB / 1.2 + 370
                observe("act", nc.scalar.activation(
                    d["oT"][:], d["op"][:], AF.Identity, bias=trb1[:, 0:1]))

            def s_out():
                onp_ = pA.tile([128, 64], f32, tag="pA")
                for b4 in range(4):
                    observe("pe", nc.tensor.transpose(
                        onp_[:, 16 * b4:16 * (b4 + 1)],
                        d["oT"][:, 128 * b4:128 * (b4 + 1)],
                        tid[0:16, 0:16]))
                load["dve"] += 64 / 0.96 + 250
                observe("dve", nc.vector.tensor_copy(
                    oN[:, 64 * sb:64 * (sb + 1)], onp_[:]))

            def s_store():
                yv = y.rearrange("(b p) f -> p b f", p=128)
                nc.gpsimd.dma_start(
                    yv[:, 4 * sb:4 * (sb + 1), :],
                    oN[:, 64 * sb:64 * (sb + 1)].rearrange(
                        "p (b f) -> p b f", f=16))

            def s_out_store():
                s_out()
                s_store()

            deferred.extend([s_sg, s_summ])
            tail[sb] = [s_rho0, s_relu0, s_rho1, s_bias, s_out_store]

        for p in range(n_pairs + 2):
            if p < n_pairs:
                emit_l0(p)
            if 1 <= p < n_pairs + 1:
                emit_l1(p - 1)
            if p >= 2:
                emit_l2(p - 2)
                if (p - 2) % 16 == 15:
                    emit_rho((p - 2) // 16)
            if deferred:
                deferred.pop(0)()
        while deferred:
            deferred.pop(0)()
        for stage in range(5):
            for sb in range(n_sb):
                tail[sb][stage]()

    nc.compile()
    return nc


def prep_inputs(inputs):
    """Host-side layout prep: transposed/quantized x + packed weights."""
    f = np.float32

    def q8(a):
        return np.asarray(a, f).astype(F8NP)

    w0 = np.asarray(inputs["phi_w0"], f) * W0SCALE   # [16, 128]
    wAf = np.zeros((128, 2048), f)
    for m in range(8):
        var = np.zeros((128, 128), f)
        var[16 * m:16 * m + 16, :] = w0
        hi = q8(var)
        res = q8(var - hi.astype(f))
        wAf[:, 256 * m:256 * m + 128] = hi.astype(f)
        wAf[:, 256 * m + 128:256 * m + 256] = res.astype(f)
    wA = wAf.astype(F8NP)

    rho_w0 = np.asarray(inputs["rho_w0"], f)
    rho_w0 = np.concatenate([rho_w0[32:96], rho_w0[0:32]], axis=0)
    rho_w1 = np.asarray(inputs["rho_w1"], f)
    partsD = {
        "w1": np.asarray(inputs["phi_w1"], f),
        "w2": np.asarray(inputs["phi_w2"], f),
        "rw0a": rho_w0[:, :128],
        "rw0b": rho_w0[:, 128:],
        "rw1a": rho_w1[:128],
        "rw1b": rho_w1[128:],
    }
    partsC = {
        "b0s": (W0SCALE * np.asarray(inputs["phi_b0"], f)).reshape(128, 1),
        "b1s": (W0SCALE * np.asarray(inputs["phi_b1"], f)).reshape(128, 1),
        "b2s": (N_NEIGH * np.asarray(inputs["phi_b2"], f)).reshape(HIDDEN, 1),
        "rb0a": np.asarray(inputs["rho_b0"], f)[:128].reshape(128, 1),
        "rb0b": np.asarray(inputs["rho_b0"], f)[128:].reshape(128, 1),
        "rb1": np.asarray(inputs["rho_b1"], f).reshape(16, 1),
        "ident": np.eye(16, dtype=f),
    }
    wCm = np.zeros((128, WC_COLS), f)
    for name, (c0, c1, p) in _WOFF_C.items():
        arr = partsC[name]
        assert arr.shape == (p, c1 - c0), (name, arr.shape)
        wCm[:p, c0:c1] = arr
    wDm = np.zeros((128, WD_COLS), f)
    for name, (c0, c1, p) in _WOFF_D.items():
        arr = partsD[name]
        assert arr.shape == (p, c1 - c0), (name, arr.shape)
        wDm[:p, c0:c1] = arr
    wts = {"wA": wA, "wC": wCm, "wD": wDm}

    x = np.asarray(inputs["x"], f)
    assert x.shape == (B_FULL, XCOLS)
    in_maps = []
    for c in range(N_CORES):
        xT = np.ascontiguousarray(x[c * BC:(c + 1) * BC].T)  # [544, BC]
        in_maps.append({
            "xs8": np.ascontiguousarray(xT[32:544]).astype(F8NP),
            "xg": np.ascontiguousarray(xT[0:32]),
            **wts,
        })
    return in_maps


def prep_weights(inputs):  # kept for test.py compatibility
    return prep_inputs(inputs)[0]


def kernel(**inputs):
    from concourse.bass_utils import run_bass_kernel_spmd

    if "nc" not in _CACHE:
        _CACHE["nc"] = build_nc(NSB_FULL)
    nc = _CACHE["nc"]

    in_maps = prep_inputs(inputs)
    res = run_bass_kernel_spmd(nc, in_maps, list(range(N_CORES)))
    out = np.concatenate([res.results[c]["y"] for c in range(N_CORES)], axis=0)
    return out.astype(np.float32)


# revision 19
# speedup vs baseline: 1.0155x; 1.0155x over previous
"""DeepSet (segment_reduce) Trainium2 Bass kernel, v2.

Computes, for each batch row b of x [B, 544]:
    s_i = x[:, :16]; s_g = x[:, 16:32]; s_js = x[:, 32:].reshape(B, 32, 16)
    h   = relu(s_js @ W0 + b0); h = relu(h @ W1 + b1); h = h @ W2 + b2
    summ = h.sum(axis=1)
    out = relu([s_i, s_g, summ] @ RW0 + rb0) @ RW1 + rb1        # [B, 16]

Sharding: pure data-parallel over 8 NeuronCores (batch 16384 -> 8 x 2048),
weights replicated.

v2 structure (vs the v1 transpose-on-chip design):
- x is transposed on the HOST: neighbor features arrive as xs8 [512, 2048]
  fp8e4m3 (slab-major feature rows x batch cols) and s_i/s_g as xg [32, 2048]
  f32. No PE transposes, no PSUM->SBUF x copies.
- L0 runs as fp8 DoubleRow matmuls (0.5 PE cycles/row): stationary packs
  [q8(8*W0var) | q8-residual] planes, the moving slab is a stride-0
  broadcast, so the residual plane corrects the weight quantization for
  free. h0 PSUM = 8*(W0^T x8); biases are pre-scaled 8x on the host and the
  8x is divided out at the summ drain (exact power-of-two arithmetic).
- L1/L2/rho stay float32r (full accuracy; L2 neighbor-sum fused via PSUM
  accumulation).
- The two big PSUM->SBUF relu drains per pair are split greedily between
  ACT and DVE (Pool/GPSIMD cannot access PSUM on TRN2).
"""

import numpy as np
import ml_dtypes
from contextlib import ExitStack

F8NP = ml_dtypes.float8_e4m3

STATE_DIM = 16
N_NEIGH = 32
HIDDEN = 64
XCOLS = (2 + N_NEIGH) * STATE_DIM  # 544
B_FULL = 16384
N_CORES = 8
BC = B_FULL // N_CORES  # 2048 rows per core
SB = 512                # batch rows per super-block (matmul N)
NSB_FULL = BC // SB     # 4
W0SCALE = 8.0

_CACHE = {}

_WOFF_C = {
    "b0s": (0, 1, 128),
    "b1s": (1, 2, 128),
    "b2s": (2, 3, HIDDEN),
    "rb0a": (3, 4, 128),
    "rb0b": (4, 5, 128),
    "rb1": (5, 6, 16),
    "ident": (6, 22, 16),
}
WC_COLS = 22
_WOFF_D = {
    "w1": (0, 128, 128),
    "w2": (128, 192, 128),
    "rw0a": (192, 320, 96),
    "rw0b": (320, 448, 96),
    "rw1a": (448, 464, 128),
    "rw1b": (464, 480, 128),
}
WD_COLS = 480


def build_nc(n_sb=NSB_FULL):
    import concourse.bass as bass
    import concourse.bacc as bacc
    import concourse.tile as tile
    import concourse.mybir as mybir

    f32 = mybir.dt.float32
    f32r = mybir.dt.float32r
    f8 = mybir.dt.float8e4
    AF = mybir.ActivationFunctionType
    ALU = mybir.AluOpType
    DR = mybir.MatmulPerfMode.DoubleRow

    rows = n_sb * SB
    n_blocks = rows // 128
    nc = bacc.Bacc("TRN2", target_bir_lowering=False, debug=False)

    xs8 = nc.declare_dram_parameter("xs8", [512, rows], f8, isOutput=False)
    xg = nc.declare_dram_parameter("xg", [32, rows], f32, isOutput=False)
    wA = nc.declare_dram_parameter("wA", [128, 2048], f8, isOutput=False)
    wC = nc.declare_dram_parameter("wC", [128, WC_COLS], f32, isOutput=False)
    wD = nc.declare_dram_parameter("wD", [128, WD_COLS], f32r, isOutput=False)
    y = nc.declare_dram_parameter("y", [rows, 16], f32, isOutput=True)

    with tile.TileContext(nc) as tc, ExitStack() as ctx:
        wp = ctx.enter_context(tc.tile_pool(name="wts", bufs=1))
        # DMA-written tiles get dedicated slots (single-sync-wait rule).
        pxs = ctx.enter_context(tc.tile_pool(name="xs", bufs=4 * n_sb))
        ph0 = ctx.enter_context(tc.tile_pool(name="h0", bufs=6))
        pr1 = ctx.enter_context(tc.tile_pool(name="r1", bufs=6))
        pX96 = ctx.enter_context(tc.tile_pool(name="X96", bufs=n_sb))
        pr0 = ctx.enter_context(tc.tile_pool(name="r0", bufs=n_sb))
        poT = ctx.enter_context(tc.tile_pool(name="oT", bufs=n_sb))
        poN = ctx.enter_context(tc.tile_pool(name="oN", bufs=1))
        pA = ctx.enter_context(tc.tile_pool(name="pA", bufs=3, space="PSUM"))
        qAcc = ctx.enter_context(tc.tile_pool(name="qAcc", bufs=2, space="PSUM"))

        twC = wp.tile([128, WC_COLS], f32, tag="wC")
        dmaC = nc.sync.dma_start(twC[:], wC[:])
        twA0 = wp.tile([128, 2048], f8, tag="wA0")
        dmaA = nc.sync.dma_start(twA0[:], wA[:])

        xst = {}

        def load_xs(sb):
            for j in range(4):
                t = pxs.tile([128, SB], f8, tag="xs", name=f"xs{sb}_{j}")
                nc.sync.dma_start(
                    t[:], xs8[128 * j:128 * (j + 1), SB * sb:SB * (sb + 1)])
                xst[(sb, j)] = t

        twD = wp.tile([128, WD_COLS], f32r, tag="wD")
        dmaD = nc.sync.dma_start(twD[:], wD[:])
        load_xs(0)
        if n_sb > 1:
            load_xs(1)
        txg = wp.tile([32, rows], f32, tag="xg")
        dmaG = nc.sync.dma_start(txg[:], xg[:])
        for sb in range(2, n_sb):
            load_xs(sb)

        def wc(name):
            c0, c1, p = _WOFF_C[name]
            return twC[0:p, c0:c1]

        def wd(name):
            c0, c1, p = _WOFF_D[name]
            return twD[0:p, c0:c1]

        def w0var(m):  # [128, 2, 128] fp8 hi|res planes of 8*W0 variant m
            return twA0[:, 256 * m:256 * (m + 1)].rearrange(
                "p (two c) -> p two c", two=2)

        tw1, tw2 = wd("w1"), wd("w2")
        trw0a, trw0b = wd("rw0a"), wd("rw0b")
        trw1a, trw1b = wd("rw1a"), wd("rw1b")
        tb0, tb1, tb2s = wc("b0s"), wc("b1s"), wc("b2s")
        trb0a, trb0b, trb1 = wc("rb0a"), wc("rb0b"), wc("rb1")
        tid = wc("ident")

        # Single-sync-wait discipline: each engine observes the startup DMAs
        # it depends on through dummy single-wait ops before real work.
        prev = {"pe": None, "act": None, "dve": None}

        def observe(k, ins):
            if prev[k] is not None:
                tile.add_dep_helper(ins.ins, prev[k].ins, sync=False,
                                    reason="startup order")
            prev[k] = ins

        dq0 = qAcc.tile([1, 1], f32, tag="qAcc")
        observe("pe", nc.tensor.matmul(
            dq0[0:1, 0:1], twC[0:1, 0:1], twC[0:1, 0:1],
            start=True, stop=True))
        dq2 = qAcc.tile([1, 1], f32, tag="qAcc")
        observe("pe", nc.tensor.matmul(
            dq2[0:1, 0:1], twA0[0:1, 0:4].bitcast(f32),
            twA0[0:1, 0:4].bitcast(f32), start=True, stop=True))
        da0 = wp.tile([1, 1], f32, tag="dumA0")
        observe("act", nc.scalar.copy(da0[0:1, 0:1], twC[0:1, 0:1]))
        dv0 = wp.tile([1, 1], f32, tag="dumV0")
        observe("dve", nc.vector.tensor_copy(dv0[0:1, 0:1], twC[0:1, 0:1]))

        # Greedy ACT/DVE load balance for the big PSUM relu drains.
        load = {"act": 0.0, "dve": 0.0}

        def drain_relu(dst, src, bias, eng=None):
            n = src.shape[-1]
            c_act = n / 1.2 + 370
            c_dve = n / 0.92 + 250
            if eng is None:
                eng = ("act" if load["act"] + c_act <= load["dve"] + c_dve
                       else "dve")
            if eng == "act":
                load["act"] += c_act
                observe("act", nc.scalar.activation(
                    dst, src, AF.Relu, bias=bias))
            else:
                load["dve"] += c_dve
                observe("dve", nc.vector.tensor_scalar(
                    dst, src, bias, 0.0, ALU.add, ALU.max))

        n_pairs = 16 * n_sb
        state = {}
        sbs = {}
        deferred = []
        tail = {}
        oN = poN.tile([128, 16 * n_blocks], f32, tag="oN")

        def emit_l0(p):
            sb = p // 16
            n0 = 2 * (p % 16)
            if p % 16 == 0:
                X96 = pX96.tile([96, SB], f32r, tag="X96", name=f"X96_{sb}")
                sbs[sb] = {"X96": X96, "acc": None}
            hp = pA.tile([128, 1024], f32, tag="pA")
            for k in (0, 1):
                n = n0 + k
                observe("pe", nc.tensor.matmul(
                    hp[:, 512 * k:512 * (k + 1)],
                    w0var(n % 8),
                    xst[(sb, n // 8)][:].unsqueeze(1).broadcast_to(
                        [128, 2, SB]),
                    start=True, stop=True, perf_mode=DR,
                ))
            h0s = ph0.tile([128, 1024], f32r, tag="h0")
            drain_relu(h0s[:], hp[:], tb0[:, 0:1], eng="dve")
            state[p] = {"h0s": h0s, "hp": hp}

        def emit_l1(p):
            if p == 0:
                dq1 = qAcc.tile([1, 1], f32, tag="qAcc")
                observe("pe", nc.tensor.matmul(
                    dq1[0:1, 0:1], twD[0:1, 0:1].bitcast(f32),
                    twD[0:1, 0:1].bitcast(f32), start=True, stop=True))
            h0s = state[p]["h0s"]
            h1p = state[p]["hp"]  # reuse: WAR == the existing h0s RAW dep
            for k in (0, 1):
                observe("pe", nc.tensor.matmul(
                    h1p[:, 512 * k:512 * (k + 1)], tw1,
                    h0s[:, 512 * k:512 * (k + 1)],
                    start=True, stop=True,
                ))
            r1 = pr1.tile([128, 1024], f32r, tag="r1")
            drain_relu(r1[:], h1p[:], tb1[:, 0:1], eng="act")
            state[p]["r1"] = r1

        def emit_l2(p):
            sb = p // 16
            n0 = 2 * (p % 16)
            if sbs[sb]["acc"] is None:
                sbs[sb]["acc"] = qAcc.tile([HIDDEN, SB], f32, tag="qAcc",
                                           name=f"acc{sb}")
            acc = sbs[sb]["acc"]
            r1 = state[p]["r1"]
            for k in (0, 1):
                n = n0 + k
                observe("pe", nc.tensor.matmul(
                    acc[:], tw2, r1[:, 512 * k:512 * (k + 1)],
                    start=(n == 0), stop=(n == N_NEIGH - 1),
                ))
            del state[p]

        def emit_rho(sb):
            d = sbs[sb]

            def s_sg():
                load["act"] += SB / 1.2 + 370
                observe("act", nc.scalar.copy(
                    d["X96"][64:96, :], txg[:, SB * sb:SB * (sb + 1)]))

            def s_summ():
                load["act"] += SB / 1.2 + 370
                observe("act", nc.scalar.activation(
                    d["X96"][0:64, :], d["acc"][:], AF.Identity,
                    bias=tb2s[:, 0:1], scale=1.0 / W0SCALE))

            def s_rho0():
                d["r0p"] = pA.tile([128, 1024], f32, tag="pA", name=f"r0p{sb}")
                observe("pe", nc.tensor.matmul(
                    d["r0p"][:, 0:512], trw0a, d["X96"][:],
                    start=True, stop=True))
                observe("pe", nc.tensor.matmul(
                    d["r0p"][:, 512:1024], trw0b, d["X96"][:],
                    start=True, stop=True))

            def s_relu0():
                d["r0s"] = pr0.tile([128, 1024], f32r, tag="r0",
                                    name=f"r0s{sb}")
                observe("act", nc.scalar.activation(
                    d["r0s"][:, 0:512], d["r0p"][:, 0:512], AF.Relu,
                    bias=trb0a[:, 0:1]))
                observe("dve", nc.vector.tensor_scalar(
                    d["r0s"][:, 512:1024], d["r0p"][:, 512:1024],
                    trb0b[:, 0:1], 0.0, ALU.add, ALU.max))

            def s_rho1():
                d["op"] = pA.tile([16, SB], f32, tag="pA", name=f"op{sb}")
                observe("pe", nc.tensor.matmul(
                    d["op"][:], trw1a, d["r0s"][:, 0:512],
                    start=True, stop=False))
                observe("pe", nc.tensor.matmul(
                    d["op"][:], trw1b, d["r0s"][:, 512:1024],
                    start=False, stop=True))

            def s_bias():
                d["oT"] = poT.tile([16, SB], f32, tag="oT", name=f"oT{sb}")
                load["act"] += SB / 1.2 + 370
                observe("act", nc.scalar.activation(
                    d["oT"][:], d["op"][:], AF.Identity, bias=trb1[:, 0:1]))

            def s_out():
                onp_ = pA.tile([128, 64], f32, tag="pA")
                for b4 in range(4):
                    observe("pe", nc.tensor.transpose(
                        onp_[:, 16 * b4:16 * (b4 + 1)],
                        d["oT"][:, 128 * b4:128 * (b4 + 1)],
                        tid[0:16, 0:16]))
                load["dve"] += 64 / 0.96 + 250
                observe("dve", nc.vector.tensor_copy(
                    oN[:, 64 * sb:64 * (sb + 1)], onp_[:]))

            def s_store():
                yv = y.rearrange("(b p) f -> p b f", p=128)
                nc.gpsimd.dma_start(
                    yv[:, 4 * sb:4 * (sb + 1), :],
                    oN[:, 64 * sb:64 * (sb + 1)].rearrange(
                        "p (b f) -> p b f", f=16))

            def s_out_store():
                s_out()
                s_store()

            deferred.extend([s_sg, s_summ])
            tail[sb] = [s_rho0, s_relu0, s_rho1, s_bias, s_out_store]

        for p in range(n_pairs + 2):
            if p < n_pairs:
                emit_l0(p)
            if 1 <= p < n_pairs + 1:
                emit_l1(p - 1)
            if p >= 2:
                emit_l2(p - 2)
                if (p - 2) % 16 == 15:
                    emit_rho((p - 2) // 16)
            if deferred:
                deferred.pop(0)()
        while deferred:
            deferred.pop(0)()
        for stage in range(5):
            for sb in range(n_sb):
                tail[sb][stage]()

    nc.compile()
    return nc


def prep_inputs(inputs):
    """Host-side layout prep: transposed/quantized x + packed weights."""
    f = np.float32

    def q8(a):
        return np.asarray(a, f).astype(F8NP)

    w0 = np.asarray(inputs["phi_w0"], f) * W0SCALE   # [16, 128]
    wAf = np.zeros((128, 2048), f)
    for m in range(8):
        var = np.zeros((128, 128), f)
        var[16 * m:16 * m + 16, :] = w0
        hi = q8(var)
        res = q8(var - hi.astype(f))
        wAf[:, 256 * m:256 * m + 128] = hi.astype(f)
        wAf[:, 256 * m + 128:256 * m + 256] = res.astype(f)
    wA = wAf.astype(F8NP)

    rho_w0 = np.asarray(inputs["rho_w0"], f)
    rho_w0 = np.concatenate([rho_w0[32:96], rho_w0[0:32]], axis=0)
    rho_w1 = np.asarray(inputs["rho_w1"], f)
    partsD = {
        "w1": np.asarray(inputs["phi_w1"], f),
        "w2": np.asarray(inputs["phi_w2"], f),
        "rw0a": rho_w0[:, :128],
        "rw0b": rho_w0[:, 128:],
        "rw1a": rho_w1[:128],
        "rw1b": rho_w1[128:],
    }
    partsC = {
        "b0s": (W0SCALE * np.asarray(inputs["phi_b0"], f)).reshape(128, 1),
        "b1s": (W0SCALE * np.asarray(inputs["phi_b1"], f)).reshape(128, 1),
        "b2s": (N_NEIGH * np.asarray(inputs["phi_b2"], f)).reshape(HIDDEN, 1),
        "rb0a": np.asarray(inputs["rho_b0"], f)[:128].reshape(128, 1),
        "rb0b": np.asarray(inputs["rho_b0"], f)[128:].reshape(128, 1),
        "rb1": np.asarray(inputs["rho_b1"], f).reshape(16, 1),
        "ident": np.eye(16, dtype=f),
    }
    wCm = np.zeros((128, WC_COLS), f)
    for name, (c0, c1, p) in _WOFF_C.items():
        arr = partsC[name]
        assert arr.shape == (p, c1 - c0), (name, arr.shape)
        wCm[:p, c0:c1] = arr
    wDm = np.zeros((128, WD_COLS), f)
    for name, (c0, c1, p) in _WOFF_D.items():
        arr = partsD[name]
        assert arr.shape == (p, c1 - c0), (name, arr.shape)
        wDm[:p, c0:c1] = arr
    wts = {"wA": wA, "wC": wCm, "wD": wDm}

    x = np.asarray(inputs["x"], f)
    assert x.shape == (B_FULL, XCOLS)
    in_maps = []
    for c in range(N_CORES):
        xT = np.ascontiguousarray(x[c * BC:(c + 1) * BC].T)  # [544, BC]
        in_maps.append({
            "xs8": np.ascontiguousarray(xT[32:544]).astype(F8NP),
            "xg": np.ascontiguousarray(xT[0:32]),
            **wts,
        })
    return in_maps


def prep_weights(inputs):  # kept for test.py compatibility
    return prep_inputs(inputs)[0]


def kernel(**inputs):
    from concourse.bass_utils import run_bass_kernel_spmd

    if "nc" not in _CACHE:
        _CACHE["nc"] = build_nc(NSB_FULL)
    nc = _CACHE["nc"]

    in_maps = prep_inputs(inputs)
    res = run_bass_kernel_spmd(nc, in_maps, list(range(N_CORES)))
    out = np.concatenate([res.results[c]["y"] for c in range(N_CORES)], axis=0)
    return out.astype(np.float32)


# revision 20
# speedup vs baseline: 1.0373x; 1.0214x over previous
"""DeepSet (segment_reduce) Trainium2 Bass kernel, v2.

Computes, for each batch row b of x [B, 544]:
    s_i = x[:, :16]; s_g = x[:, 16:32]; s_js = x[:, 32:].reshape(B, 32, 16)
    h   = relu(s_js @ W0 + b0); h = relu(h @ W1 + b1); h = h @ W2 + b2
    summ = h.sum(axis=1)
    out = relu([s_i, s_g, summ] @ RW0 + rb0) @ RW1 + rb1        # [B, 16]

Sharding: pure data-parallel over 8 NeuronCores (batch 16384 -> 8 x 2048),
weights replicated.

v2 structure (vs the v1 transpose-on-chip design):
- x is transposed on the HOST: neighbor features arrive as xs8 [512, 2048]
  fp8e4m3 (slab-major feature rows x batch cols) and s_i/s_g as xg [32, 2048]
  f32. No PE transposes, no PSUM->SBUF x copies.
- L0 runs as fp8 DoubleRow matmuls (0.5 PE cycles/row): stationary packs
  [q8(8*W0var) | q8-residual] planes, the moving slab is a stride-0
  broadcast, so the residual plane corrects the weight quantization for
  free. h0 PSUM = 8*(W0^T x8); biases are pre-scaled 8x on the host and the
  8x is divided out at the summ drain (exact power-of-two arithmetic).
- L1/L2/rho stay float32r (full accuracy; L2 neighbor-sum fused via PSUM
  accumulation).
- The two big PSUM->SBUF relu drains per pair are split greedily between
  ACT and DVE (Pool/GPSIMD cannot access PSUM on TRN2).
"""

import numpy as np
import ml_dtypes
from contextlib import ExitStack

F8NP = ml_dtypes.float8_e4m3

STATE_DIM = 16
N_NEIGH = 32
HIDDEN = 64
XCOLS = (2 + N_NEIGH) * STATE_DIM  # 544
B_FULL = 16384
N_CORES = 8
BC = B_FULL // N_CORES  # 2048 rows per core
SB = 512                # batch rows per super-block (matmul N)
NSB_FULL = BC // SB     # 4
W0SCALE = 8.0

_CACHE = {}

_WOFF_C = {
    "b0s": (0, 1, 128),
    "b1s": (1, 2, 128),
    "b2s": (2, 3, HIDDEN),
    "rb0a": (3, 4, 128),
    "rb0b": (4, 5, 128),
    "rb1": (5, 6, 16),
    "ident": (6, 22, 16),
}
WC_COLS = 22
_WOFF_D = {
    "w1": (0, 128, 128),
    "w2": (128, 192, 128),
    "rw0a": (192, 320, 96),
    "rw0b": (320, 448, 96),
    "rw1a": (448, 464, 128),
    "rw1b": (464, 480, 128),
}
WD_COLS = 480


def build_nc(n_sb=NSB_FULL):
    import concourse.bass as bass
    import concourse.bacc as bacc
    import concourse.tile as tile
    import concourse.mybir as mybir

    f32 = mybir.dt.float32
    f32r = mybir.dt.float32r
    f8 = mybir.dt.float8e4
    AF = mybir.ActivationFunctionType
    ALU = mybir.AluOpType
    DR = mybir.MatmulPerfMode.DoubleRow

    rows = n_sb * SB
    n_blocks = rows // 128
    nc = bacc.Bacc("TRN2", target_bir_lowering=False, debug=False)

    xs8 = nc.declare_dram_parameter("xs8", [512, rows], f8, isOutput=False)
    xg = nc.declare_dram_parameter("xg", [32, rows], f32, isOutput=False)
    wA = nc.declare_dram_parameter("wA", [128, 2048], f8, isOutput=False)
    wC = nc.declare_dram_parameter("wC", [128, WC_COLS], f32, isOutput=False)
    wD = nc.declare_dram_parameter("wD", [128, WD_COLS], f32r, isOutput=False)
    y = nc.declare_dram_parameter("y", [rows, 16], f32, isOutput=True)

    with tile.TileContext(nc) as tc, ExitStack() as ctx:
        wp = ctx.enter_context(tc.tile_pool(name="wts", bufs=1))
        # DMA-written tiles get dedicated slots (single-sync-wait rule).
        pxs = ctx.enter_context(tc.tile_pool(name="xs", bufs=4 * n_sb))
        ph0 = ctx.enter_context(tc.tile_pool(name="h0", bufs=6))
        pr1 = ctx.enter_context(tc.tile_pool(name="r1", bufs=6))
        pX96 = ctx.enter_context(tc.tile_pool(name="X96", bufs=n_sb))
        pr0 = ctx.enter_context(tc.tile_pool(name="r0", bufs=n_sb))
        poT = ctx.enter_context(tc.tile_pool(name="oT", bufs=n_sb))
        poN = ctx.enter_context(tc.tile_pool(name="oN", bufs=1))
        pA = ctx.enter_context(tc.tile_pool(name="pA", bufs=3, space="PSUM"))
        qAcc = ctx.enter_context(tc.tile_pool(name="qAcc", bufs=2, space="PSUM"))

        twC = wp.tile([128, WC_COLS], f32, tag="wC")
        dmaC = nc.sync.dma_start(twC[:], wC[:])
        twA0 = wp.tile([128, 2048], f8, tag="wA0")
        dmaA = nc.sync.dma_start(twA0[:], wA[:])

        xst = {}

        def load_xs(sb):
            for j in range(4):
                t = pxs.tile([128, SB], f8, tag="xs", name=f"xs{sb}_{j}")
                nc.sync.dma_start(
                    t[:], xs8[128 * j:128 * (j + 1), SB * sb:SB * (sb + 1)])
                xst[(sb, j)] = t

        twD = wp.tile([128, WD_COLS], f32r, tag="wD")
        dmaD = nc.sync.dma_start(twD[:], wD[:])
        load_xs(0)
        if n_sb > 1:
            load_xs(1)
        txg = wp.tile([32, rows], f32, tag="xg")
        dmaG = nc.sync.dma_start(txg[:], xg[:])
        for sb in range(2, n_sb):
            load_xs(sb)

        def wc(name):
            c0, c1, p = _WOFF_C[name]
            return twC[0:p, c0:c1]

        def wd(name):
            c0, c1, p = _WOFF_D[name]
            return twD[0:p, c0:c1]

        def w0var(m):  # [128, 2, 128] fp8 hi|res planes of 8*W0 variant m
            return twA0[:, 256 * m:256 * (m + 1)].rearrange(
                "p (two c) -> p two c", two=2)

        tw1, tw2 = wd("w1"), wd("w2")
        trw0a, trw0b = wd("rw0a"), wd("rw0b")
        trw1a, trw1b = wd("rw1a"), wd("rw1b")
        tb0, tb1, tb2s = wc("b0s"), wc("b1s"), wc("b2s")
        trb0a, trb0b, trb1 = wc("rb0a"), wc("rb0b"), wc("rb1")
        tid = wc("ident")

        # Single-sync-wait discipline: each engine observes the startup DMAs
        # it depends on through dummy single-wait ops before real work.
        prev = {"pe": None, "act": None, "dve": None}

        def observe(k, ins):
            if prev[k] is not None:
                tile.add_dep_helper(ins.ins, prev[k].ins, sync=False,
                                    reason="startup order")
            prev[k] = ins

        dq0 = qAcc.tile([1, 1], f32, tag="qAcc")
        observe("pe", nc.tensor.matmul(
            dq0[0:1, 0:1], twC[0:1, 0:1], twC[0:1, 0:1],
            start=True, stop=True))
        dq2 = qAcc.tile([1, 1], f32, tag="qAcc")
        observe("pe", nc.tensor.matmul(
            dq2[0:1, 0:1], twA0[0:1, 0:4].bitcast(f32),
            twA0[0:1, 0:4].bitcast(f32), start=True, stop=True))
        da0 = wp.tile([1, 1], f32, tag="dumA0")
        observe("act", nc.scalar.copy(da0[0:1, 0:1], twC[0:1, 0:1]))
        dv0 = wp.tile([1, 1], f32, tag="dumV0")
        observe("dve", nc.vector.tensor_copy(dv0[0:1, 0:1], twC[0:1, 0:1]))

        # Greedy ACT/DVE load balance for the big PSUM relu drains.
        load = {"act": 0.0, "dve": 0.0}

        def drain_relu(dst, src, bias, eng=None):
            n = src.shape[-1]
            c_act = n / 1.2 + 370
            c_dve = n / 0.92 + 250
            if eng is None:
                eng = ("act" if load["act"] + c_act <= load["dve"] + c_dve
                       else "dve")
            if eng == "act":
                load["act"] += c_act
                observe("act", nc.scalar.activation(
                    dst, src, AF.Relu, bias=bias))
            else:
                load["dve"] += c_dve
                observe("dve", nc.vector.tensor_scalar(
                    dst, src, bias, 0.0, ALU.add, ALU.max))

        n_pairs = 16 * n_sb
        state = {}
        sbs = {}
        deferred = []
        tail = {}
        oN = poN.tile([128, 16 * n_blocks], f32, tag="oN")

        def emit_l0(p):
            sb = p // 16
            n0 = 2 * (p % 16)
            if p % 16 == 0:
                X96 = pX96.tile([96, SB], f32r, tag="X96", name=f"X96_{sb}")
                sbs[sb] = {"X96": X96, "acc": None}
            hp = pA.tile([128, 1024], f32, tag="pA")
            for k in (0, 1):
                n = n0 + k
                observe("pe", nc.tensor.matmul(
                    hp[:, 512 * k:512 * (k + 1)],
                    w0var(n % 8),
                    xst[(sb, n // 8)][:].unsqueeze(1).broadcast_to(
                        [128, 2, SB]),
                    start=True, stop=True, perf_mode=DR,
                ))
            h0s = ph0.tile([128, 1024], f32r, tag="h0")
            drain_relu(h0s[:], hp[:], tb0[:, 0:1], eng="dve")
            state[p] = {"h0s": h0s, "hp": hp}

        def emit_l1(p):
            if p == 0:
                dq1 = qAcc.tile([1, 1], f32, tag="qAcc")
                observe("pe", nc.tensor.matmul(
                    dq1[0:1, 0:1], twD[0:1, 0:1].bitcast(f32),
                    twD[0:1, 0:1].bitcast(f32), start=True, stop=True))
            h0s = state[p]["h0s"]
            h1p = state[p]["hp"]  # reuse: WAR == the existing h0s RAW dep
            for k in (0, 1):
                observe("pe", nc.tensor.matmul(
                    h1p[:, 512 * k:512 * (k + 1)], tw1,
                    h0s[:, 512 * k:512 * (k + 1)],
                    start=True, stop=True,
                ))
            r1 = pr1.tile([128, 1024], f32r, tag="r1")
            drain_relu(r1[:], h1p[:], tb1[:, 0:1], eng="act")
            state[p]["r1"] = r1

        def emit_l2(p):
            sb = p // 16
            n0 = 2 * (p % 16)
            if sbs[sb]["acc"] is None:
                sbs[sb]["acc"] = qAcc.tile([HIDDEN, SB], f32, tag="qAcc",
                                           name=f"acc{sb}")
            acc = sbs[sb]["acc"]
            r1 = state[p]["r1"]
            for k in (0, 1):
                n = n0 + k
                observe("pe", nc.tensor.matmul(
                    acc[:], tw2, r1[:, 512 * k:512 * (k + 1)],
                    start=(n == 0), stop=(n == N_NEIGH - 1),
                ))
            del state[p]

        def emit_rho(sb):
            d = sbs[sb]

            def s_sg():
                load["act"] += SB / 1.2 + 370
                observe("act", nc.scalar.copy(
                    d["X96"][64:96, :], txg[:, SB * sb:SB * (sb + 1)]))

            def s_summ():
                load["act"] += SB / 1.2 + 370
                observe("act", nc.scalar.activation(
                    d["X96"][0:64, :], d["acc"][:], AF.Identity,
                    bias=tb2s[:, 0:1], scale=1.0 / W0SCALE))

            def s_rho0():
                d["r0p"] = pA.tile([128, 1024], f32, tag="pA", name=f"r0p{sb}")
                observe("pe", nc.tensor.matmul(
                    d["r0p"][:, 0:512], trw0a, d["X96"][:],
                    start=True, stop=True))
                observe("pe", nc.tensor.matmul(
                    d["r0p"][:, 512:1024], trw0b, d["X96"][:],
                    start=True, stop=True))

            def s_relu0():
                d["r0s"] = pr0.tile([128, 1024], f32r, tag="r0",
                                    name=f"r0s{sb}")
                observe("act", nc.scalar.activation(
                    d["r0s"][:, 0:512], d["r0p"][:, 0:512], AF.Relu,
                    bias=trb0a[:, 0:1]))
                observe("dve", nc.vector.tensor_scalar(
                    d["r0s"][:, 512:1024], d["r0p"][:, 512:1024],
                    trb0b[:, 0:1], 0.0, ALU.add, ALU.max))

            def s_rho1():
                d["op"] = pA.tile([16, SB], f32, tag="pA", name=f"op{sb}")
                observe("pe", nc.tensor.matmul(
                    d["op"][:], trw1a, d["r0s"][:, 0:512],
                    start=True, stop=False))
                observe("pe", nc.tensor.matmul(
                    d["op"][:], trw1b, d["r0s"][:, 512:1024],
                    start=False, stop=True))

            def s_bias():
                d["oT"] = poT.tile([16, SB], f32, tag="oT", name=f"oT{sb}")
                load["act"] += SB / 1.2 + 370
                observe("act", nc.scalar.activation(
                    d["oT"][:], d["op"][:], AF.Identity, bias=trb1[:, 0:1]))

            def s_out():
                onp_ = pA.tile([128, 64], f32, tag="pA")
                for b4 in range(4):
                    observe("pe", nc.tensor.transpose(
                        onp_[:, 16 * b4:16 * (b4 + 1)],
                        d["oT"][:, 128 * b4:128 * (b4 + 1)],
                        tid[0:16, 0:16]))
                load["dve"] += 64 / 0.96 + 250
                observe("dve", nc.vector.tensor_copy(
                    oN[:, 64 * sb:64 * (sb + 1)], onp_[:]))

            def s_store():
                yv = y.rearrange("(b p) f -> p b f", p=128)
                nc.sync.dma_start(
                    yv[:, 4 * sb:4 * (sb + 1), :],
                    oN[:, 64 * sb:64 * (sb + 1)].rearrange(
                        "p (b f) -> p b f", f=16))

            def s_out_store():
                s_out()
                s_store()

            deferred.extend([s_sg, s_summ])
            tail[sb] = [s_rho0, s_relu0, s_rho1, s_bias, s_out_store]

        for p in range(n_pairs + 2):
            if p < n_pairs:
                emit_l0(p)
            if 1 <= p < n_pairs + 1:
                emit_l1(p - 1)
            if p >= 2:
                emit_l2(p - 2)
                if (p - 2) % 16 == 15:
                    emit_rho((p - 2) // 16)
            if deferred:
                deferred.pop(0)()
        while deferred:
            deferred.pop(0)()
        for stage in range(5):
            for sb in range(n_sb):
                tail[sb][stage]()

    nc.compile()
    return nc


def prep_inputs(inputs):
    """Host-side layout prep: transposed/quantized x + packed weights."""
    f = np.float32

    def q8(a):
        return np.asarray(a, f).astype(F8NP)

    w0 = np.asarray(inputs["phi_w0"], f) * W0SCALE   # [16, 128]
    wAf = np.zeros((128, 2048), f)
    for m in range(8):
        var = np.zeros((128, 128), f)
        var[16 * m:16 * m + 16, :] = w0
        hi = q8(var)
        res = q8(var - hi.astype(f))
        wAf[:, 256 * m:256 * m + 128] = hi.astype(f)
        wAf[:, 256 * m + 128:256 * m + 256] = res.astype(f)
    wA = wAf.astype(F8NP)

    rho_w0 = np.asarray(inputs["rho_w0"], f)
    rho_w0 = np.concatenate([rho_w0[32:96], rho_w0[0:32]], axis=0)
    rho_w1 = np.asarray(inputs["rho_w1"], f)
    partsD = {
        "w1": np.asarray(inputs["phi_w1"], f),
        "w2": np.asarray(inputs["phi_w2"], f),
        "rw0a": rho_w0[:, :128],
        "rw0b": rho_w0[:, 128:],
        "rw1a": rho_w1[:128],
        "rw1b": rho_w1[128:],
    }
    partsC = {
        "b0s": (W0SCALE * np.asarray(inputs["phi_b0"], f)).reshape(128, 1),
        "b1s": (W0SCALE * np.asarray(inputs["phi_b1"], f)).reshape(128, 1),
        "b2s": (N_NEIGH * np.asarray(inputs["phi_b2"], f)).reshape(HIDDEN, 1),
        "rb0a": np.asarray(inputs["rho_b0"], f)[:128].reshape(128, 1),
        "rb0b": np.asarray(inputs["rho_b0"], f)[128:].reshape(128, 1),
        "rb1": np.asarray(inputs["rho_b1"], f).reshape(16, 1),
        "ident": np.eye(16, dtype=f),
    }
    wCm = np.zeros((128, WC_COLS), f)
    for name, (c0, c1, p) in _WOFF_C.items():
        arr = partsC[name]
        assert arr.shape == (p, c1 - c0), (name, arr.shape)
        wCm[:p, c0:c1] = arr
    wDm = np.zeros((128, WD_COLS), f)
    for name, (c0, c1, p) in _WOFF_D.items():
        arr = partsD[name]
        assert arr.shape == (p, c1 - c0), (name, arr.shape)
        wDm[:p, c0:c1] = arr
    wts = {"wA": wA, "wC": wCm, "wD": wDm}

    x = np.asarray(inputs["x"], f)
    assert x.shape == (B_FULL, XCOLS)
    in_maps = []
    for c in range(N_CORES):
        xT = np.ascontiguousarray(x[c * BC:(c + 1) * BC].T)  # [544, BC]
        in_maps.append({
            "xs8": np.ascontiguousarray(xT[32:544]).astype(F8NP),
            "xg": np.ascontiguousarray(xT[0:32]),
            **wts,
        })
    return in_maps


def prep_weights(inputs):  # kept for test.py compatibility
    return prep_inputs(inputs)[0]


def kernel(**inputs):
    from concourse.bass_utils import run_bass_kernel_spmd

    if "nc" not in _CACHE:
        _CACHE["nc"] = build_nc(NSB_FULL)
    nc = _CACHE["nc"]

    in_maps = prep_inputs(inputs)
    res = run_bass_kernel_spmd(nc, in_maps, list(range(N_CORES)))
    out = np.concatenate([res.results[c]["y"] for c in range(N_CORES)], axis=0)
    return out.astype(np.float32)
